# revision 1
# baseline (speedup 1.0000x reference)
"""Trainium2 Bass kernel for the CML1D problem.

Math: 15 steps of  g' = bdm + 0.5 - (q0*s[i-1] + q1*s[i] + q2*s[i+1]),
with s = (g-0.5)^2, folded so the device loop per step is:
  psum = W^T s          (banded fp32 matmul, lattice on partitions)
  u    = psum + bdm     (DVE scalar_tensor_tensor; bdm = beta*drive + const - 0.5)
  s'   = u^2            (ACT Square)
Layout: lattice split into overlapping windows of 128 (stride 118, halo 5 per
side) on the partition dim; batch on the free dim. Halo margin 5 means only two
SBUF halo-refill exchanges (after steps 5 and 10). Host does the windowing /
unwindowing (pure data marshalling); all arithmetic runs on device. State is
held in per-column-group tiles so cross-step dependencies stay group-local.
"""
import sys

sys.path.insert(0, "/opt/trn_rl_repo")
from contextlib import ExitStack

import numpy as np

import concourse.tile as tile
from concourse import bacc, mybir
from concourse.bass_utils import run_bass_kernel_spmd

F32 = mybir.dt.float32
AF = mybir.ActivationFunctionType
OP = mybir.AluOpType

R, EPS, BETA, STEPS = 3.9, 0.3, 0.15, 15
CLIP_LO, CLIP_HI = 0.0001, 1.0 - 0.0001

P = 128          # partitions / window size
H = 5            # halo per side
S = P - 2 * H    # window stride = 118
N_CORES = 8

LAT = 16384
BATCH = 2048
BPC = BATCH // N_CORES   # 256 rows per core
BB = 64                  # batch rows per block
NBLK = BPC // BB

GROUP = 1024             # psum drain group (2 banks)
MMN = 512                # max fp32 moving free dim per matmul


def _nw(lat):
    nw = -(-lat // S)
    assert lat - S * (nw - 1) >= H, "ragged seam too small for halo exchange"
    return nw


def build_nc(lat=LAT, bb=BB, nblk=NBLK, steps=STEPS):
    """Build the per-core Bass module (identical on all cores)."""
    nw = _nw(lat)
    cols = nw * bb
    seam = lat - S * (nw - 1)  # partition offset of seam-window wrap source
    nc = bacc.Bacc("TRN2", target_bir_lowering=False, debug=False)
    drive_w = nc.dram_tensor("drive_w", [nblk, P, cols], F32, kind="ExternalInput")
    wmat = nc.dram_tensor("wmat", [P, P], F32, kind="ExternalInput")
    consts = nc.dram_tensor("consts", [P, 2], F32, kind="ExternalInput")
    out_w = nc.dram_tensor("out_w", [nblk, S, cols], F32, kind="ExternalOutput")

    # column group boundaries (psum drain granularity)
    groups = []
    g0 = 0
    while g0 < cols:
        groups.append((g0, min(g0 + GROUP, cols)))
        g0 = min(g0 + GROUP, cols)
    ng = len(groups)

    # exchange DMAs spread across several queue engines
    def dma_engines(ncb):
        return [ncb.sync, ncb.scalar, ncb.gpsimd]

    with tile.TileContext(nc) as tc, ExitStack() as ctx:
        const_pool = ctx.enter_context(tc.tile_pool(name="constp", bufs=1))
        sp = ctx.enter_context(tc.tile_pool(name="state", bufs=2))
        up = ctx.enter_context(tc.tile_pool(name="u", bufs=3))
        lp = ctx.enter_context(tc.tile_pool(name="ld", bufs=3))
        pp = ctx.enter_context(tc.tile_pool(name="ps", bufs=4, space="PSUM"))

        w_t = const_pool.tile([P, P], F32, tag="w")
        nc.sync.dma_start(w_t[:], wmat.ap())
        c_t = const_pool.tile([P, 2], F32, tag="c")
        nc.sync.dma_start(c_t[:], consts.ap())
        beta_ap = c_t[:, 0:1]
        csum_ap = c_t[:, 1:2]
        neg_half = const_pool.tile([P, 1], F32, tag="nh")
        nc.vector.memset(neg_half[:], -0.5)

        for blk in range(nblk):
            s = sp.tile([P, cols], F32, tag="s", name=f"s_b{blk}")
            bdm = sp.tile([P, cols], F32, tag="bdm", name=f"bdm_b{blk}")

            # load drive per group into staging; derive s0 and bdm (loads on
            # gpsimd so next block's loads aren't queued behind stores)
            for gi, (g0, g1) in enumerate(groups):
                gw = g1 - g0
                dt_ = lp.tile([P, gw], F32, tag="ld", name=f"ld{gi}_b{blk}")
                nc.gpsimd.dma_start(dt_[:], drive_w.ap()[blk, :, g0:g1])
                nc.scalar.activation(
                    s[:, g0:g1], dt_[:], AF.Square, bias=neg_half[:], scale=1.0
                )
                nc.vector.tensor_scalar(
                    bdm[:, g0:g1], dt_[:], beta_ap, csum_ap, op0=OP.mult, op1=OP.add
                )

            for t in range(steps):
                last = t == steps - 1
                if (t + 1) % H == 0 and not last:
                    order = [ng - 1] + list(range(ng - 1))
                else:
                    order = list(range(ng))
                for gi in order:
                    g0, g1 = groups[gi]
                    gw = g1 - g0
                    pt = pp.tile([P, gw], F32, tag="ps", name=f"pt{t}_{gi}_b{blk}")
                    m0 = 0
                    while m0 < gw:
                        m1 = min(m0 + MMN, gw)
                        nc.tensor.matmul(
                            pt[:, m0:m1],
                            w_t[:],
                            s[:, g0 + m0 : g0 + m1],
                            start=True,
                            stop=True,
                        )
                        m0 = m1
                    ut = up.tile([P, gw], F32, tag="u", name=f"ut{t}_{gi}_b{blk}")
                    if not last:
                        nc.vector.scalar_tensor_tensor(
                            ut[:], pt[:], 1.0, bdm[:, g0:g1], op0=OP.mult, op1=OP.add
                        )
                        nc.scalar.activation(
                            s[:, g0:g1], ut[:], AF.Square, bias=0.0, scale=1.0
                        )
                    else:
                        # g = psum + 0.5 + bdm, then clip, then store
                        nc.vector.scalar_tensor_tensor(
                            ut[:], pt[:], 0.5, bdm[:, g0:g1], op0=OP.add, op1=OP.add
                        )
                        u2 = up.tile([P, gw], F32, tag="u", name=f"u2{t}_{gi}_b{blk}")
                        nc.vector.tensor_scalar(
                            u2[:],
                            ut[:],
                            CLIP_LO,
                            CLIP_HI,
                            op0=OP.max,
                            op1=OP.min,
                        )
                        nc.sync.dma_start(
                            out_w.ap()[blk, :, g0:g1], u2[H : H + S, :]
                        )

                if (t + 1) % H == 0 and not last:
                    # halo refill: seams first (dep only on seam group, drained
                    # first), then E1/E3 split in halves so early copies only
                    # depend on early drains
                    lastw = nw - 1
                    p0 = S * (nw - 1) - lat + P - H
                    nc.sync.dma_start(
                        s[0:H, 0:bb], s[seam : seam + H, lastw * bb : (lastw + 1) * bb]
                    )
                    nc.gpsimd.dma_start(
                        s[P - H : P, lastw * bb : (lastw + 1) * bb],
                        s[p0 : p0 + H, 0:bb],
                    )
                    mid = (ng // 2) * (GROUP // bb) * bb  # col split point
                    if mid == 0 or mid >= (nw - 1) * bb:
                        mid = None
                    # E1: left halos of windows 1..nw-1 <- prev window
                    if mid:
                        nc.sync.dma_start(
                            s[0:H, bb : mid + bb], s[P - 2 * H : P - H, 0:mid]
                        )
                        nc.sync.dma_start(
                            s[0:H, mid + bb : nw * bb],
                            s[P - 2 * H : P - H, mid : (nw - 1) * bb],
                        )
                        nc.gpsimd.dma_start(
                            s[P - H : P, 0:mid], s[H : 2 * H, bb : mid + bb]
                        )
                        nc.gpsimd.dma_start(
                            s[P - H : P, mid : (nw - 1) * bb],
                            s[H : 2 * H, mid + bb : nw * bb],
                        )
                    else:
                        nc.sync.dma_start(
                            s[0:H, bb : nw * bb],
                            s[P - 2 * H : P - H, 0 : (nw - 1) * bb],
                        )
                        nc.gpsimd.dma_start(
                            s[P - H : P, 0 : (nw - 1) * bb],
                            s[H : 2 * H, bb : nw * bb],
                        )

    nc.compile()
    return nc


def _host_constants(K):
    K = np.asarray(K, dtype=np.float64)
    q0 = (1.0 - BETA) * EPS * K[0] * R
    q1 = (1.0 - BETA) * (1.0 - EPS + EPS * K[1]) * R
    q2 = (1.0 - BETA) * EPS * K[2] * R
    W = np.zeros((P, P), np.float32)
    for p in range(1, P - 1):
        W[p - 1, p] = -q0
        W[p, p] = -q1
        W[p + 1, p] = -q2
    csum = 0.25 * (q0 + q1 + q2) - 0.5
    consts = np.empty((P, 2), np.float32)
    consts[:, 0] = BETA
    consts[:, 1] = csum
    return W, consts


def _window(d, lat, bb, nblk):
    """[rows, lat] -> [nblk, P, nw*bb] with halo windows (stride S)."""
    nw = _nw(lat)
    c_idx = np.arange(nw) * S
    p_idx = np.arange(P)
    idx = (c_idx[:, None] + p_idx[None, :] - H) % lat  # [nw, P]
    win = d[:, idx]  # [rows, nw, P]
    win = win.reshape(nblk, bb, nw, P).transpose(0, 3, 2, 1)  # [nblk, P, nw, bb]
    return np.ascontiguousarray(win).reshape(nblk, P, nw * bb)


def _unwindow(o, lat, bb, nblk):
    """[nblk, S, nw*bb] -> [rows, lat]."""
    nw = _nw(lat)
    o = o.reshape(nblk, S, nw, bb).transpose(0, 3, 2, 1)  # [nblk, bb, nw, S]
    o = o.reshape(nblk * bb, nw * S)
    return o[:, :lat]


_NC_CACHE = {}
TRACE = False
LAST_RESULT = None


def _get_nc(lat, bb, nblk, steps):
    key = (lat, bb, nblk, steps)
    if key not in _NC_CACHE:
        _NC_CACHE[key] = build_nc(lat, bb, nblk, steps)
    return _NC_CACHE[key]


def kernel(drive, K):
    drive = np.asarray(drive, dtype=np.float32)
    K = np.asarray(K, dtype=np.float32)
    b, mid, lat = drive.shape
    d2 = drive.reshape(b, lat)
    W, consts = _host_constants(K)
    nc = _get_nc(LAT, BB, NBLK, STEPS)

    in_maps = []
    for c in range(N_CORES):
        dcore = d2[c * BPC : (c + 1) * BPC]
        in_maps.append(
            {
                "drive_w": _window(dcore, LAT, BB, NBLK),
                "wmat": W,
                "consts": consts,
            }
        )
    global LAST_RESULT
    res = None
    for attempt in range(3):
        try:
            res = run_bass_kernel_spmd(
                nc, in_maps, core_ids=list(range(N_CORES)), trace=TRACE
            )
            break
        except Exception:
            # transient NRT device errors have been observed; reset the jax
            # backend (forces a fresh PJRT client / device session) and retry
            if attempt == 2:
                raise
            import time

            try:
                import jax

                jax.clear_caches()
                from jax._src import xla_bridge

                xla_bridge._clear_backends()
            except Exception:
                pass
            time.sleep(5.0)
    LAST_RESULT = res
    outs = [_unwindow(res.results[c]["out_w"], LAT, BB, NBLK) for c in range(N_CORES)]
    out = np.concatenate(outs, axis=0).reshape(b, mid, lat).astype(np.float32)
    return out



# revision 3
# speedup vs baseline: 1.6644x; 1.6644x over previous
"""Trainium2 Bass kernel for the CML1D problem — balanced 4-engine version.

Math: 15 steps of  u = W^T s + beta*drive + csum;  s' = u^2, with
s = (g-0.5)^2, W the negated banded conv matrix, csum = 0.25*sum(q)-0.5.
Final g = u + 0.5 (clipped on host).

Per step, per column group, one of two pipelines:

  B-groups (cols [0, BCOLS); drive enters via the PE):
    psum = W^T s + I^T hi + I^T lo   (3 f32r matmuls per 512-chunk;
                                      hi/lo = grid-exact split of beta*drive)
    s'   = Square(psum + csum)       (ACT, bias = const, psum -> sbuf f32r)

  A-groups (drive enters via host-precomputed bdm = beta*drive + csum):
    psum = W^T s                     (banded f32r matmul)
    u    = psum + bdm                (scalar_tensor_tensor, DVE)
    s'   = u^2                       (ACT Square or Pool tensor_tensor)

float32r matmuls run 1 cycle/row (vs 4 for fp32); f32r keeps 11 mantissa
bits (RNE), costing ~8e-3 end-to-end rel err vs the 2e-2 budget. The hi/lo
split keeps the drive term exact so the B path adds no extra error.

Layout: lattice in overlapping 128-site windows (stride 118, halo 5/side)
on partitions; (window, batch-row) pairs on the free dim. Halo refill every
5 steps via on-chip DMAs. Host does windowing, s0/bdm/hi/lo precompute,
unwindowing, and the final clip.
"""
import sys

sys.path.insert(0, "/opt/trn_rl_repo")
from contextlib import ExitStack

import numpy as np

import concourse.tile as tile
from concourse import bacc, mybir
from concourse.bass_utils import run_bass_kernel_spmd

F32 = mybir.dt.float32
F32R = mybir.dt.float32r
AF = mybir.ActivationFunctionType
OP = mybir.AluOpType

R, EPS, BETA, STEPS = 3.9, 0.3, 0.15, 15
CLIP_LO, CLIP_HI = 0.0001, 1.0 - 0.0001

P = 128          # partitions / window size
H = 5            # halo per side
S = P - 2 * H    # window stride = 118
N_CORES = 8

LAT = 16384
BATCH = 2048
BPC = BATCH // N_CORES   # 256 rows per core
BB = 32                  # batch rows per block
NBLK = BPC // BB

GROUP = 1024             # column group (2 psum banks)
BCOLS = 2048             # cols [0, BCOLS) use the drive-via-matmul B path
SQ_ACT_HI = 3072         # A-cols [BCOLS, SQ_ACT_HI) squared on ACT; rest Pool


def _nw(lat):
    nw = -(-lat // S)
    assert lat - S * (nw - 1) >= H, "ragged seam too small for halo exchange"
    return nw


def build_nc(lat=LAT, bb=BB, nblk=NBLK, steps=STEPS, bcols=BCOLS,
             sq_act_hi=SQ_ACT_HI, group=GROUP, psum_bufs=4, u_bufs=4,
             order_mode="a_first", stt_split=None, sp_bufs=4):
    """Build the per-core Bass module (identical on all cores)."""
    nw = _nw(lat)
    cols = nw * bb
    seam = lat - S * (nw - 1)  # partition offset of seam-window wrap source
    acols = cols - bcols
    nc = bacc.Bacc("TRN2", target_bir_lowering=False, debug=False)
    s0_w = nc.dram_tensor("s0_w", [nblk, P, cols], F32R, kind="ExternalInput")
    bdm_w = nc.dram_tensor("bdm_w", [nblk, P, acols], F32, kind="ExternalInput")
    drv_b = nc.dram_tensor("drv_b", [nblk, P, 2 * bcols], F32R, kind="ExternalInput")
    wmat = nc.dram_tensor("wmat", [P, P], F32R, kind="ExternalInput")
    dmat = nc.dram_tensor("dmat", [P, P], F32R, kind="ExternalInput")
    consts = nc.dram_tensor("consts", [P, 2], F32, kind="ExternalInput")
    out_w = nc.dram_tensor("out_w", [nblk, S, cols], F32, kind="ExternalOutput")

    groups = []
    g0 = 0
    while g0 < cols:
        groups.append((g0, min(g0 + group, cols)))
        g0 = min(g0 + group, cols)
    ng = len(groups)
    assert bcols < groups[-1][0], "seam group must stay on the A path"
    assert bcols % group == 0, "B region must end on a group boundary"

    with tile.TileContext(nc) as tc, ExitStack() as ctx:
        const_pool = ctx.enter_context(tc.tile_pool(name="constp", bufs=1))
        sp = ctx.enter_context(tc.tile_pool(name="state", bufs=sp_bufs))
        up = ctx.enter_context(tc.tile_pool(name="u", bufs=u_bufs))
        pp = ctx.enter_context(tc.tile_pool(name="ps", bufs=psum_bufs, space="PSUM"))

        w_t = const_pool.tile([P, P], F32R, tag="w")
        nc.sync.dma_start(w_t[:], wmat.ap())
        d_t = const_pool.tile([P, P], F32R, tag="d")
        nc.sync.dma_start(d_t[:], dmat.ap())
        c_t = const_pool.tile([P, 2], F32, tag="c")
        nc.sync.dma_start(c_t[:], consts.ap())
        csum_ap = c_t[:, 0:1]     # csum = 0.25*(q0+q1+q2) - 0.5
        csum_h_ap = c_t[:, 1:2]   # csum + 0.5 (for B-group last step)

        block_tiles = {}

        def load_block(b):
            s_ = sp.tile([P, cols], F32R, tag="s", name=f"s_b{b}")
            nc.sync.dma_start(s_[:], s0_w.ap()[b])
            bdm_ = sp.tile([P, acols], F32, tag="bdm", name=f"bdm_b{b}")
            nc.sync.dma_start(bdm_[:], bdm_w.ap()[b])
            dr_ = sp.tile([P, 2 * bcols], F32R, tag="dr", name=f"dr_b{b}")
            nc.sync.dma_start(dr_[:], drv_b.ap()[b])
            block_tiles[b] = (s_, bdm_, dr_)

        nb_g = bcols // group
        if order_mode == "a_first":
            base_order = list(range(nb_g, ng)) + list(range(nb_g))
        else:
            base_order = list(range(ng))

        def emit_step(blk, t):
            s, bdm, dr = block_tiles[blk]
            last = t == steps - 1
            if (t + 1) % H == 0 and not last:
                order = [ng - 1] + [g for g in base_order if g != ng - 1]
            else:
                order = base_order
            for gi in order:
                    g0, g1 = groups[gi]
                    gw = g1 - g0
                    is_b = g1 <= bcols
                    pt = pp.tile([P, gw], F32, tag="ps", name=f"pt{t}_{gi}_b{blk}")
                    for m0 in range(0, gw, 512):
                        m1 = min(m0 + 512, gw)
                        nc.tensor.matmul(
                            pt[:, m0:m1], w_t[:], s[:, g0 + m0 : g0 + m1],
                            start=True, stop=not is_b,
                        )
                        if is_b:
                            nc.tensor.matmul(
                                pt[:, m0:m1], d_t[:], dr[:, g0 + m0 : g0 + m1],
                                start=False, stop=False,
                            )
                            nc.tensor.matmul(
                                pt[:, m0:m1], d_t[:],
                                dr[:, bcols + g0 + m0 : bcols + g0 + m1],
                                start=False, stop=True,
                            )
                    if is_b:
                        if not last:
                            nc.scalar.activation(
                                s[:, g0:g1], pt[:], AF.Square,
                                bias=csum_ap, scale=1.0,
                            )
                        else:
                            ut = up.tile([P, gw], F32, tag="u", name=f"ut{t}_{gi}_b{blk}")
                            nc.scalar.activation(
                                ut[:], pt[:], AF.Identity, bias=csum_h_ap, scale=1.0
                            )
                            nc.sync.dma_start(
                                out_w.ap()[blk, :, g0:g1], ut[H : H + S, :]
                            )
                        continue
                    # --- A-group ---
                    ut = up.tile([P, gw], F32, tag="u", name=f"ua{t}_{gi}_b{blk}")
                    if not last:
                        is_pool = g0 >= sq_act_hi
                        # Pool groups: 512-wide half-chains (mm half -> STT
                        # half -> square half) to break the serial per-step
                        # dependency cycle; others stay full-width.
                        sw = stt_split or (512 if is_pool and gw > 512 else gw)
                        for s0_ in range(0, gw, sw):
                            s1_ = min(s0_ + sw, gw)
                            nc.vector.scalar_tensor_tensor(
                                ut[:, s0_:s1_], pt[:, s0_:s1_], 1.0,
                                bdm[:, g0 + s0_ - bcols : g0 + s1_ - bcols],
                                op0=OP.mult, op1=OP.add,
                            )
                            if not is_pool:
                                nc.scalar.activation(
                                    s[:, g0 + s0_ : g0 + s1_], ut[:, s0_:s1_],
                                    AF.Square, bias=0.0, scale=1.0,
                                )
                            else:
                                nc.gpsimd.tensor_tensor(
                                    s[:, g0 + s0_ : g0 + s1_], ut[:, s0_:s1_],
                                    ut[:, s0_:s1_], op=OP.mult,
                                )
                    else:
                        # g-0.5... host adds nothing: u already = g (incl +0.5)
                        nc.vector.scalar_tensor_tensor(
                            ut[:], pt[:], 0.5, bdm[:, g0 - bcols : g1 - bcols],
                            op0=OP.add, op1=OP.add,
                        )
                        nc.sync.dma_start(
                            out_w.ap()[blk, :, g0:g1], ut[H : H + S, :]
                        )

        def emit_xchg(blk):
            # halo refill: seams first (dep only on seam group, drained
            # first), then E1/E3 split in halves so early copies only depend
            # on early drains. DMAs spread across SP and ACT queues so their
            # issue+DGE latencies overlap.
            s, _, _ = block_tiles[blk]
            lastw = nw - 1
            p0 = S * (nw - 1) - lat + P - H
            nc.sync.dma_start(
                s[0:H, 0:bb], s[seam : seam + H, lastw * bb : (lastw + 1) * bb]
            )
            nc.sync.dma_start(
                s[P - H : P, lastw * bb : (lastw + 1) * bb],
                s[p0 : p0 + H, 0:bb],
            )
            mid = (ng // 2) * (group // bb) * bb  # col split point
            if mid == 0 or mid >= (nw - 1) * bb:
                mid = None
            if mid:
                nc.sync.dma_start(
                    s[0:H, bb : mid + bb], s[P - 2 * H : P - H, 0:mid]
                )
                nc.sync.dma_start(
                    s[P - H : P, 0:mid], s[H : 2 * H, bb : mid + bb]
                )
                nc.sync.dma_start(
                    s[0:H, mid + bb : nw * bb],
                    s[P - 2 * H : P - H, mid : (nw - 1) * bb],
                )
                nc.sync.dma_start(
                    s[P - H : P, mid : (nw - 1) * bb],
                    s[H : 2 * H, mid + bb : nw * bb],
                )
            else:
                nc.sync.dma_start(
                    s[0:H, bb : nw * bb],
                    s[P - 2 * H : P - H, 0 : (nw - 1) * bb],
                )
                nc.sync.dma_start(
                    s[P - H : P, 0 : (nw - 1) * bb],
                    s[H : 2 * H, bb : nw * bb],
                )

        # Paired-block interleave: steps of blocks (2k, 2k+1) alternate so
        # one block's halo-exchange barrier (and per-group dependency cycles)
        # are hidden under the other block's compute.
        assert nblk % 2 == 0
        load_block(0)
        load_block(1)
        for pair in range(nblk // 2):
            a, b = 2 * pair, 2 * pair + 1
            if 2 * pair + 2 < nblk:
                load_block(2 * pair + 2)  # slot free: block 2*pair-2 is done
            for t in range(steps):
                emit_step(a, t)
                if (t + 1) % H == 0 and t != steps - 1:
                    emit_xchg(a)
                emit_step(b, t)
                if (t + 1) % H == 0 and t != steps - 1:
                    emit_xchg(b)
            if 2 * pair + 3 < nblk:
                load_block(2 * pair + 3)  # after pair's stores are queued
            del block_tiles[a], block_tiles[b]

    nc.compile()
    return nc


def _round_f32r_grid(x):
    """Round float32 array to the f32r grid (11 mantissa bits, round-half-up).
    Any value with the low 12 mantissa bits zero is exactly representable, so
    the device DMA's own f32r rounding is then the identity."""
    u = np.ascontiguousarray(x, dtype=np.float32).view(np.uint32)
    u2 = (u + np.uint32(1 << 11)) & np.uint32(0xFFFFF000)
    return u2.view(np.float32)


def _host_constants(K):
    K = np.asarray(K, dtype=np.float64)
    q0 = (1.0 - BETA) * EPS * K[0] * R
    q1 = (1.0 - BETA) * (1.0 - EPS + EPS * K[1]) * R
    q2 = (1.0 - BETA) * EPS * K[2] * R
    W = np.zeros((P, P), np.float32)
    for p in range(1, P - 1):
        W[p - 1, p] = -q0
        W[p, p] = -q1
        W[p + 1, p] = -q2
    D = np.zeros((P, P), np.float32)
    np.fill_diagonal(D, 1.0)
    csum = 0.25 * (q0 + q1 + q2) - 0.5
    consts = np.empty((P, 2), np.float32)
    consts[:, 0] = csum
    consts[:, 1] = csum + 0.5
    return W, D, consts


def _window(d, lat, bb, nblk):
    """[rows, lat] -> [nblk, P, nw*bb] with halo windows (stride S)."""
    nw = _nw(lat)
    c_idx = np.arange(nw) * S
    p_idx = np.arange(P)
    idx = (c_idx[:, None] + p_idx[None, :] - H) % lat  # [nw, P]
    win = d[:, idx]  # [rows, nw, P]
    win = win.reshape(nblk, bb, nw, P).transpose(0, 3, 2, 1)  # [nblk, P, nw, bb]
    return np.ascontiguousarray(win).reshape(nblk, P, nw * bb)


def _unwindow(o, lat, bb, nblk):
    """[nblk, S, nw*bb] -> [rows, lat]."""
    nw = _nw(lat)
    o = o.reshape(nblk, S, nw, bb).transpose(0, 3, 2, 1)  # [nblk, bb, nw, S]
    o = o.reshape(nblk * bb, nw * S)
    return o[:, :lat]


_NC_CACHE = {}
TRACE = False
LAST_RESULT = None


def _get_nc(lat, bb, nblk, steps):
    key = (lat, bb, nblk, steps)
    if key not in _NC_CACHE:
        _NC_CACHE[key] = build_nc(lat, bb, nblk, steps)
    return _NC_CACHE[key]


def kernel(drive, K):
    drive = np.asarray(drive, dtype=np.float32)
    K = np.asarray(K, dtype=np.float32)
    b, mid, lat = drive.shape
    d2 = drive.reshape(b, lat)
    W, D, consts = _host_constants(K)
    csum = np.float64(consts[0, 0])
    nc = _get_nc(LAT, BB, NBLK, STEPS)

    in_maps = []
    for c in range(N_CORES):
        dcore = d2[c * BPC : (c + 1) * BPC]
        dw = _window(dcore, LAT, BB, NBLK).astype(np.float64)
        s0 = ((dw - 0.5) ** 2).astype(np.float32)
        bdm = (BETA * dw[:, :, BCOLS:] + csum).astype(np.float32)
        bd = BETA * dw[:, :, :BCOLS]
        hi = _round_f32r_grid(bd.astype(np.float32))
        lo = (bd - hi.astype(np.float64)).astype(np.float32)
        drv_b = np.concatenate([hi, lo], axis=2)
        in_maps.append(
            {
                "s0_w": s0,
                "bdm_w": bdm,
                "drv_b": drv_b,
                "wmat": W,
                "dmat": D,
                "consts": consts,
            }
        )
    global LAST_RESULT
    res = None
    for attempt in range(3):
        try:
            res = run_bass_kernel_spmd(
                nc, in_maps, core_ids=list(range(N_CORES)), trace=TRACE
            )
            break
        except Exception:
            # transient NRT device errors have been observed; reset the jax
            # backend (forces a fresh PJRT client / device session) and retry
            if attempt == 2:
                raise
            import time

            try:
                import jax

                jax.clear_caches()
                from jax._src import xla_bridge

                xla_bridge._clear_backends()
            except Exception:
                pass
            time.sleep(5.0)
    LAST_RESULT = res
    outs = [_unwindow(res.results[c]["out_w"], LAT, BB, NBLK) for c in range(N_CORES)]
    out = np.concatenate(outs, axis=0).reshape(b, mid, lat)
    return np.clip(out, CLIP_LO, CLIP_HI).astype(np.float32)


# revision 4
# speedup vs baseline: 1.6860x; 1.0130x over previous
"""Trainium2 Bass kernel for the CML1D problem — balanced 4-engine version.

Math: 15 steps of  u = W^T s + beta*drive + csum;  s' = u^2, with
s = (g-0.5)^2, W the negated banded conv matrix, csum = 0.25*sum(q)-0.5.
Final g = u + 0.5 (clipped on host).

Per step, per column group, one of two pipelines:

  B-groups (cols [0, BCOLS); drive enters via the PE):
    psum = W^T s + I^T hi + I^T lo   (3 f32r matmuls per 512-chunk;
                                      hi/lo = grid-exact split of beta*drive)
    s'   = Square(psum + csum)       (ACT, bias = const, psum -> sbuf f32r)

  A-groups (drive enters via host-precomputed bdm = beta*drive + csum):
    psum = W^T s                     (banded f32r matmul)
    u    = psum + bdm                (scalar_tensor_tensor, DVE)
    s'   = u^2                       (ACT Square or Pool tensor_tensor)

float32r matmuls run 1 cycle/row (vs 4 for fp32); f32r keeps 11 mantissa
bits (RNE), costing ~8e-3 end-to-end rel err vs the 2e-2 budget. The hi/lo
split keeps the drive term exact so the B path adds no extra error.

Layout: lattice in overlapping 128-site windows (stride 118, halo 5/side)
on partitions; (window, batch-row) pairs on the free dim. Halo refill every
5 steps via on-chip DMAs. Host does windowing, s0/bdm/hi/lo precompute,
unwindowing, and the final clip.
"""
import sys

sys.path.insert(0, "/opt/trn_rl_repo")
from contextlib import ExitStack

import numpy as np

import concourse.tile as tile
from concourse import bacc, mybir
from concourse.bass_utils import run_bass_kernel_spmd

F32 = mybir.dt.float32
F32R = mybir.dt.float32r
AF = mybir.ActivationFunctionType
OP = mybir.AluOpType

R, EPS, BETA, STEPS = 3.9, 0.3, 0.15, 15
CLIP_LO, CLIP_HI = 0.0001, 1.0 - 0.0001

P = 128          # partitions / window size
H = 5            # halo per side
S = P - 2 * H    # window stride = 118
N_CORES = 8

LAT = 16384
BATCH = 2048
BPC = BATCH // N_CORES   # 256 rows per core
BB = 32                  # batch rows per block
NBLK = BPC // BB

GROUP = 1024             # column group (2 psum banks)
BCOLS = 2048             # cols [0, BCOLS) use the drive-via-matmul B path
SQ_ACT_HI = 3072         # A-cols [BCOLS, SQ_ACT_HI) squared on ACT; rest Pool


def _nw(lat):
    nw = -(-lat // S)
    assert lat - S * (nw - 1) >= H, "ragged seam too small for halo exchange"
    return nw


def build_nc(lat=LAT, bb=BB, nblk=NBLK, steps=STEPS, bcols=BCOLS,
             sq_act_hi=SQ_ACT_HI, group=GROUP, psum_bufs=4, u_bufs=4,
             order_mode="a_first", stt_split=None, sp_bufs=4):
    """Build the per-core Bass module (identical on all cores)."""
    nw = _nw(lat)
    cols = nw * bb
    seam = lat - S * (nw - 1)  # partition offset of seam-window wrap source
    acols = cols - bcols
    nc = bacc.Bacc("TRN2", target_bir_lowering=False, debug=False)
    s0_w = nc.dram_tensor("s0_w", [nblk, P, cols], F32R, kind="ExternalInput")
    bdm_w = nc.dram_tensor("bdm_w", [nblk, P, acols], F32, kind="ExternalInput")
    drv_b = nc.dram_tensor("drv_b", [nblk, P, 2 * bcols], F32R, kind="ExternalInput")
    wmat = nc.dram_tensor("wmat", [P, P], F32R, kind="ExternalInput")
    dmat = nc.dram_tensor("dmat", [P, P], F32R, kind="ExternalInput")
    consts = nc.dram_tensor("consts", [P, 2], F32, kind="ExternalInput")
    out_w = nc.dram_tensor("out_w", [nblk, S, cols], F32, kind="ExternalOutput")

    groups = []
    g0 = 0
    while g0 < cols:
        groups.append((g0, min(g0 + group, cols)))
        g0 = min(g0 + group, cols)
    ng = len(groups)
    assert bcols < groups[-1][0], "seam group must stay on the A path"
    assert bcols % group == 0, "B region must end on a group boundary"

    with tile.TileContext(nc) as tc, ExitStack() as ctx:
        const_pool = ctx.enter_context(tc.tile_pool(name="constp", bufs=1))
        sp = ctx.enter_context(tc.tile_pool(name="state", bufs=sp_bufs))
        up = ctx.enter_context(tc.tile_pool(name="u", bufs=u_bufs))
        pp = ctx.enter_context(tc.tile_pool(name="ps", bufs=psum_bufs, space="PSUM"))

        w_t = const_pool.tile([P, P], F32R, tag="w")
        nc.sync.dma_start(w_t[:], wmat.ap())
        d_t = const_pool.tile([P, P], F32R, tag="d")
        nc.sync.dma_start(d_t[:], dmat.ap())
        c_t = const_pool.tile([P, 2], F32, tag="c")
        nc.sync.dma_start(c_t[:], consts.ap())
        csum_ap = c_t[:, 0:1]     # csum = 0.25*(q0+q1+q2) - 0.5
        csum_h_ap = c_t[:, 1:2]   # csum + 0.5 (for B-group last step)

        block_tiles = {}

        def load_block(b):
            s_ = sp.tile([P, cols], F32R, tag="s", name=f"s_b{b}")
            nc.sync.dma_start(s_[:], s0_w.ap()[b])
            bdm_ = sp.tile([P, acols], F32, tag="bdm", name=f"bdm_b{b}")
            nc.sync.dma_start(bdm_[:], bdm_w.ap()[b])
            dr_ = sp.tile([P, 2 * bcols], F32R, tag="dr", name=f"dr_b{b}")
            nc.sync.dma_start(dr_[:], drv_b.ap()[b])
            block_tiles[b] = (s_, bdm_, dr_)

        nb_g = bcols // group
        if order_mode == "a_first":
            base_order = list(range(nb_g, ng)) + list(range(nb_g))
        else:
            base_order = list(range(ng))

        def emit_step(blk, t):
            s, bdm, dr = block_tiles[blk]
            last = t == steps - 1
            if (t + 1) % H == 0 and not last:
                order = [ng - 1] + [g for g in base_order if g != ng - 1]
            else:
                order = base_order
            for gi in order:
                    g0, g1 = groups[gi]
                    gw = g1 - g0
                    is_b = g1 <= bcols
                    ptag = "psb" if g1 <= bcols else "psa"
                    pt = pp.tile([P, gw], F32, tag=ptag, bufs=2,
                                 name=f"pt{t}_{gi}_b{blk}")
                    for m0 in range(0, gw, 512):
                        m1 = min(m0 + 512, gw)
                        nc.tensor.matmul(
                            pt[:, m0:m1], w_t[:], s[:, g0 + m0 : g0 + m1],
                            start=True, stop=not is_b,
                        )
                        if is_b:
                            nc.tensor.matmul(
                                pt[:, m0:m1], d_t[:], dr[:, g0 + m0 : g0 + m1],
                                start=False, stop=False,
                            )
                            nc.tensor.matmul(
                                pt[:, m0:m1], d_t[:],
                                dr[:, bcols + g0 + m0 : bcols + g0 + m1],
                                start=False, stop=True,
                            )
                    if is_b:
                        if not last:
                            nc.scalar.activation(
                                s[:, g0:g1], pt[:], AF.Square,
                                bias=csum_ap, scale=1.0,
                            )
                        else:
                            ut = up.tile([P, gw], F32, tag="u", name=f"ut{t}_{gi}_b{blk}")
                            nc.scalar.activation(
                                ut[:], pt[:], AF.Identity, bias=csum_h_ap, scale=1.0
                            )
                            nc.sync.dma_start(
                                out_w.ap()[blk, :, g0:g1], ut[H : H + S, :]
                            )
                        continue
                    # --- A-group ---
                    ut = up.tile([P, gw], F32, tag="u", name=f"ua{t}_{gi}_b{blk}")
                    if not last:
                        is_pool = g0 >= sq_act_hi
                        # Pool groups: 512-wide half-chains (mm half -> STT
                        # half -> square half) to break the serial per-step
                        # dependency cycle; others stay full-width.
                        sw = stt_split or (512 if is_pool and gw > 512 else gw)
                        for s0_ in range(0, gw, sw):
                            s1_ = min(s0_ + sw, gw)
                            nc.vector.scalar_tensor_tensor(
                                ut[:, s0_:s1_], pt[:, s0_:s1_], 1.0,
                                bdm[:, g0 + s0_ - bcols : g0 + s1_ - bcols],
                                op0=OP.mult, op1=OP.add,
                            )
                            if not is_pool:
                                nc.scalar.activation(
                                    s[:, g0 + s0_ : g0 + s1_], ut[:, s0_:s1_],
                                    AF.Square, bias=0.0, scale=1.0,
                                )
                            else:
                                nc.gpsimd.tensor_tensor(
                                    s[:, g0 + s0_ : g0 + s1_], ut[:, s0_:s1_],
                                    ut[:, s0_:s1_], op=OP.mult,
                                )
                    else:
                        # g-0.5... host adds nothing: u already = g (incl +0.5)
                        nc.vector.scalar_tensor_tensor(
                            ut[:], pt[:], 0.5, bdm[:, g0 - bcols : g1 - bcols],
                            op0=OP.add, op1=OP.add,
                        )
                        nc.sync.dma_start(
                            out_w.ap()[blk, :, g0:g1], ut[H : H + S, :]
                        )

        def emit_xchg(blk):
            # halo refill: seams first (dep only on seam group, drained
            # first), then E1/E3 split in halves so early copies only depend
            # on early drains. DMAs spread across SP and ACT queues so their
            # issue+DGE latencies overlap.
            s, _, _ = block_tiles[blk]
            lastw = nw - 1
            p0 = S * (nw - 1) - lat + P - H
            nc.sync.dma_start(
                s[0:H, 0:bb], s[seam : seam + H, lastw * bb : (lastw + 1) * bb]
            )
            nc.sync.dma_start(
                s[P - H : P, lastw * bb : (lastw + 1) * bb],
                s[p0 : p0 + H, 0:bb],
            )
            mid = (ng // 2) * (group // bb) * bb  # col split point
            if mid == 0 or mid >= (nw - 1) * bb:
                mid = None
            if mid:
                nc.sync.dma_start(
                    s[0:H, bb : mid + bb], s[P - 2 * H : P - H, 0:mid]
                )
                nc.sync.dma_start(
                    s[P - H : P, 0:mid], s[H : 2 * H, bb : mid + bb]
                )
                nc.sync.dma_start(
                    s[0:H, mid + bb : nw * bb],
                    s[P - 2 * H : P - H, mid : (nw - 1) * bb],
                )
                nc.sync.dma_start(
                    s[P - H : P, mid : (nw - 1) * bb],
                    s[H : 2 * H, mid + bb : nw * bb],
                )
            else:
                nc.sync.dma_start(
                    s[0:H, bb : nw * bb],
                    s[P - 2 * H : P - H, 0 : (nw - 1) * bb],
                )
                nc.sync.dma_start(
                    s[P - H : P, 0 : (nw - 1) * bb],
                    s[H : 2 * H, bb : nw * bb],
                )

        # Paired-block interleave: steps of blocks (2k, 2k+1) alternate so
        # one block's halo-exchange barrier (and per-group dependency cycles)
        # are hidden under the other block's compute.
        assert nblk % 2 == 0
        load_block(0)
        load_block(1)
        for pair in range(nblk // 2):
            a, b = 2 * pair, 2 * pair + 1
            if 2 * pair + 2 < nblk:
                load_block(2 * pair + 2)  # slot free: block 2*pair-2 is done
            for t in range(steps):
                emit_step(a, t)
                if (t + 1) % H == 0 and t != steps - 1:
                    emit_xchg(a)
                emit_step(b, t)
                if (t + 1) % H == 0 and t != steps - 1:
                    emit_xchg(b)
            if 2 * pair + 3 < nblk:
                load_block(2 * pair + 3)  # after pair's stores are queued
            del block_tiles[a], block_tiles[b]

    nc.compile()
    return nc


def _round_f32r_grid(x):
    """Round float32 array to the f32r grid (11 mantissa bits, round-half-up).
    Any value with the low 12 mantissa bits zero is exactly representable, so
    the device DMA's own f32r rounding is then the identity."""
    u = np.ascontiguousarray(x, dtype=np.float32).view(np.uint32)
    u2 = (u + np.uint32(1 << 11)) & np.uint32(0xFFFFF000)
    return u2.view(np.float32)


def _host_constants(K):
    K = np.asarray(K, dtype=np.float64)
    q0 = (1.0 - BETA) * EPS * K[0] * R
    q1 = (1.0 - BETA) * (1.0 - EPS + EPS * K[1]) * R
    q2 = (1.0 - BETA) * EPS * K[2] * R
    W = np.zeros((P, P), np.float32)
    for p in range(1, P - 1):
        W[p - 1, p] = -q0
        W[p, p] = -q1
        W[p + 1, p] = -q2
    D = np.zeros((P, P), np.float32)
    np.fill_diagonal(D, 1.0)
    csum = 0.25 * (q0 + q1 + q2) - 0.5
    consts = np.empty((P, 2), np.float32)
    consts[:, 0] = csum
    consts[:, 1] = csum + 0.5
    return W, D, consts


def _window(d, lat, bb, nblk):
    """[rows, lat] -> [nblk, P, nw*bb] with halo windows (stride S)."""
    nw = _nw(lat)
    c_idx = np.arange(nw) * S
    p_idx = np.arange(P)
    idx = (c_idx[:, None] + p_idx[None, :] - H) % lat  # [nw, P]
    win = d[:, idx]  # [rows, nw, P]
    win = win.reshape(nblk, bb, nw, P).transpose(0, 3, 2, 1)  # [nblk, P, nw, bb]
    return np.ascontiguousarray(win).reshape(nblk, P, nw * bb)


def _unwindow(o, lat, bb, nblk):
    """[nblk, S, nw*bb] -> [rows, lat]."""
    nw = _nw(lat)
    o = o.reshape(nblk, S, nw, bb).transpose(0, 3, 2, 1)  # [nblk, bb, nw, S]
    o = o.reshape(nblk * bb, nw * S)
    return o[:, :lat]


_NC_CACHE = {}
TRACE = False
LAST_RESULT = None


def _get_nc(lat, bb, nblk, steps):
    key = (lat, bb, nblk, steps)
    if key not in _NC_CACHE:
        _NC_CACHE[key] = build_nc(lat, bb, nblk, steps)
    return _NC_CACHE[key]


def kernel(drive, K):
    drive = np.asarray(drive, dtype=np.float32)
    K = np.asarray(K, dtype=np.float32)
    b, mid, lat = drive.shape
    d2 = drive.reshape(b, lat)
    W, D, consts = _host_constants(K)
    csum = np.float64(consts[0, 0])
    nc = _get_nc(LAT, BB, NBLK, STEPS)

    in_maps = []
    for c in range(N_CORES):
        dcore = d2[c * BPC : (c + 1) * BPC]
        dw = _window(dcore, LAT, BB, NBLK).astype(np.float64)
        s0 = ((dw - 0.5) ** 2).astype(np.float32)
        bdm = (BETA * dw[:, :, BCOLS:] + csum).astype(np.float32)
        bd = BETA * dw[:, :, :BCOLS]
        hi = _round_f32r_grid(bd.astype(np.float32))
        lo = (bd - hi.astype(np.float64)).astype(np.float32)
        drv_b = np.concatenate([hi, lo], axis=2)
        in_maps.append(
            {
                "s0_w": s0,
                "bdm_w": bdm,
                "drv_b": drv_b,
                "wmat": W,
                "dmat": D,
                "consts": consts,
            }
        )
    global LAST_RESULT
    res = None
    for attempt in range(3):
        try:
            res = run_bass_kernel_spmd(
                nc, in_maps, core_ids=list(range(N_CORES)), trace=TRACE
            )
            break
        except Exception:
            # transient NRT device errors have been observed; reset the jax
            # backend (forces a fresh PJRT client / device session) and retry
            if attempt == 2:
                raise
            import time

            try:
                import jax

                jax.clear_caches()
                from jax._src import xla_bridge

                xla_bridge._clear_backends()
            except Exception:
                pass
            time.sleep(5.0)
    LAST_RESULT = res
    outs = [_unwindow(res.results[c]["out_w"], LAT, BB, NBLK) for c in range(N_CORES)]
    out = np.concatenate(outs, axis=0).reshape(b, mid, lat)
    return np.clip(out, CLIP_LO, CLIP_HI).astype(np.float32)


# revision 5
# speedup vs baseline: 1.7030x; 1.0101x over previous
"""Trainium2 Bass kernel for the CML1D problem — balanced 4-engine version.

Math: 15 steps of  u = W^T s + beta*drive + csum;  s' = u^2, with
s = (g-0.5)^2, W the negated banded conv matrix, csum = 0.25*sum(q)-0.5.
Final g = u + 0.5 (clipped on host).

Per step, per column group, one of two pipelines:

  B-groups (cols [0, BCOLS); drive enters via the PE):
    psum = W^T s + I^T hi + I^T lo   (3 f32r matmuls per 512-chunk;
                                      hi/lo = grid-exact split of beta*drive)
    s'   = Square(psum + csum)       (ACT, bias = const, psum -> sbuf f32r)

  A-groups (drive enters via host-precomputed bdm = beta*drive + csum):
    psum = W^T s                     (banded f32r matmul)
    u    = psum + bdm                (scalar_tensor_tensor, DVE)
    s'   = u^2                       (ACT Square or Pool tensor_tensor)

float32r matmuls run 1 cycle/row (vs 4 for fp32); f32r keeps 11 mantissa
bits (RNE), costing ~8e-3 end-to-end rel err vs the 2e-2 budget. The hi/lo
split keeps the drive term exact so the B path adds no extra error.

Layout: lattice in overlapping 128-site windows (stride 118, halo 5/side)
on partitions; (window, batch-row) pairs on the free dim. Halo refill every
5 steps via on-chip DMAs. Host does windowing, s0/bdm/hi/lo precompute,
unwindowing, and the final clip.
"""
import sys

sys.path.insert(0, "/opt/trn_rl_repo")
from contextlib import ExitStack

import numpy as np

import concourse.tile as tile
from concourse import bacc, mybir
from concourse.bass_utils import run_bass_kernel_spmd

F32 = mybir.dt.float32
F32R = mybir.dt.float32r
AF = mybir.ActivationFunctionType
OP = mybir.AluOpType

R, EPS, BETA, STEPS = 3.9, 0.3, 0.15, 15
CLIP_LO, CLIP_HI = 0.0001, 1.0 - 0.0001

P = 128          # partitions / window size
H = 5            # halo per side
S = P - 2 * H    # window stride = 118
N_CORES = 8

LAT = 16384
BATCH = 2048
BPC = BATCH // N_CORES   # 256 rows per core
BB = 32                  # batch rows per block
NBLK = BPC // BB

GROUP = 1024             # column group (2 psum banks)
BCOLS = 2048             # cols [0, BCOLS) use the drive-via-matmul B path
SQ_ACT_HI = 3072         # A-cols [BCOLS, SQ_ACT_HI) squared on ACT; rest Pool


def _nw(lat):
    nw = -(-lat // S)
    assert lat - S * (nw - 1) >= H, "ragged seam too small for halo exchange"
    return nw


def build_nc(lat=LAT, bb=BB, nblk=NBLK, steps=STEPS, bcols=BCOLS,
             sq_act_hi=SQ_ACT_HI, group=GROUP, psum_bufs=4, u_bufs=4,
             order_mode="a_first", stt_split=512, sp_bufs=4):
    """Build the per-core Bass module (identical on all cores)."""
    nw = _nw(lat)
    cols = nw * bb
    seam = lat - S * (nw - 1)  # partition offset of seam-window wrap source
    acols = cols - bcols
    nc = bacc.Bacc("TRN2", target_bir_lowering=False, debug=False)
    s0_w = nc.dram_tensor("s0_w", [nblk, P, cols], F32R, kind="ExternalInput")
    bdm_w = nc.dram_tensor("bdm_w", [nblk, P, acols], F32, kind="ExternalInput")
    drv_b = nc.dram_tensor("drv_b", [nblk, P, 2 * bcols], F32R, kind="ExternalInput")
    wmat = nc.dram_tensor("wmat", [P, P], F32R, kind="ExternalInput")
    dmat = nc.dram_tensor("dmat", [P, P], F32R, kind="ExternalInput")
    consts = nc.dram_tensor("consts", [P, 2], F32, kind="ExternalInput")
    out_w = nc.dram_tensor("out_w", [nblk, S, cols], F32, kind="ExternalOutput")

    groups = []
    g0 = 0
    while g0 < cols:
        groups.append((g0, min(g0 + group, cols)))
        g0 = min(g0 + group, cols)
    ng = len(groups)
    assert bcols < groups[-1][0], "seam group must stay on the A path"
    assert bcols % group == 0, "B region must end on a group boundary"

    with tile.TileContext(nc) as tc, ExitStack() as ctx:
        const_pool = ctx.enter_context(tc.tile_pool(name="constp", bufs=1))
        sp = ctx.enter_context(tc.tile_pool(name="state", bufs=sp_bufs))
        up = ctx.enter_context(tc.tile_pool(name="u", bufs=u_bufs))
        pp = ctx.enter_context(tc.tile_pool(name="ps", bufs=psum_bufs, space="PSUM"))

        w_t = const_pool.tile([P, P], F32R, tag="w")
        nc.sync.dma_start(w_t[:], wmat.ap())
        d_t = const_pool.tile([P, P], F32R, tag="d")
        nc.sync.dma_start(d_t[:], dmat.ap())
        c_t = const_pool.tile([P, 2], F32, tag="c")
        nc.sync.dma_start(c_t[:], consts.ap())
        csum_ap = c_t[:, 0:1]     # csum = 0.25*(q0+q1+q2) - 0.5
        csum_h_ap = c_t[:, 1:2]   # csum + 0.5 (for B-group last step)

        block_tiles = {}

        def load_block(b):
            s_ = sp.tile([P, cols], F32R, tag="s", name=f"s_b{b}")
            nc.sync.dma_start(s_[:], s0_w.ap()[b])
            bdm_ = sp.tile([P, acols], F32, tag="bdm", name=f"bdm_b{b}")
            nc.sync.dma_start(bdm_[:], bdm_w.ap()[b])
            dr_ = sp.tile([P, 2 * bcols], F32R, tag="dr", name=f"dr_b{b}")
            nc.sync.dma_start(dr_[:], drv_b.ap()[b])
            block_tiles[b] = (s_, bdm_, dr_)

        nb_g = bcols // group
        if order_mode == "a_first":
            base_order = list(range(nb_g, ng)) + list(range(nb_g))
        else:
            base_order = list(range(ng))

        def emit_step(blk, t):
            s, bdm, dr = block_tiles[blk]
            last = t == steps - 1
            if (t + 1) % H == 0 and not last:
                order = [ng - 1] + [g for g in base_order if g != ng - 1]
            else:
                order = base_order
            for gi in order:
                    g0, g1 = groups[gi]
                    gw = g1 - g0
                    is_b = g1 <= bcols
                    ptag = "psb" if g1 <= bcols else "psa"
                    pt = pp.tile([P, gw], F32, tag=ptag, bufs=2,
                                 name=f"pt{t}_{gi}_b{blk}")
                    for m0 in range(0, gw, 512):
                        m1 = min(m0 + 512, gw)
                        nc.tensor.matmul(
                            pt[:, m0:m1], w_t[:], s[:, g0 + m0 : g0 + m1],
                            start=True, stop=not is_b,
                        )
                        if is_b:
                            nc.tensor.matmul(
                                pt[:, m0:m1], d_t[:], dr[:, g0 + m0 : g0 + m1],
                                start=False, stop=False,
                            )
                            nc.tensor.matmul(
                                pt[:, m0:m1], d_t[:],
                                dr[:, bcols + g0 + m0 : bcols + g0 + m1],
                                start=False, stop=True,
                            )
                    if is_b:
                        if not last:
                            nc.scalar.activation(
                                s[:, g0:g1], pt[:], AF.Square,
                                bias=csum_ap, scale=1.0,
                            )
                        else:
                            ut = up.tile([P, gw], F32, tag="u", name=f"ut{t}_{gi}_b{blk}")
                            nc.scalar.activation(
                                ut[:], pt[:], AF.Identity, bias=csum_h_ap, scale=1.0
                            )
                            nc.sync.dma_start(
                                out_w.ap()[blk, :, g0:g1], ut[H : H + S, :]
                            )
                        continue
                    # --- A-group ---
                    ut = up.tile([P, gw], F32, tag="u", name=f"ua{t}_{gi}_b{blk}")
                    if not last:
                        is_pool = g0 >= sq_act_hi
                        # Pool groups: 512-wide half-chains (mm half -> STT
                        # half -> square half) to break the serial per-step
                        # dependency cycle; others stay full-width.
                        sw = stt_split or (512 if is_pool and gw > 512 else gw)
                        for s0_ in range(0, gw, sw):
                            s1_ = min(s0_ + sw, gw)
                            nc.vector.scalar_tensor_tensor(
                                ut[:, s0_:s1_], pt[:, s0_:s1_], 1.0,
                                bdm[:, g0 + s0_ - bcols : g0 + s1_ - bcols],
                                op0=OP.mult, op1=OP.add,
                            )
                            if not is_pool:
                                nc.scalar.activation(
                                    s[:, g0 + s0_ : g0 + s1_], ut[:, s0_:s1_],
                                    AF.Square, bias=0.0, scale=1.0,
                                )
                            else:
                                nc.gpsimd.tensor_tensor(
                                    s[:, g0 + s0_ : g0 + s1_], ut[:, s0_:s1_],
                                    ut[:, s0_:s1_], op=OP.mult,
                                )
                    else:
                        # g-0.5... host adds nothing: u already = g (incl +0.5)
                        nc.vector.scalar_tensor_tensor(
                            ut[:], pt[:], 0.5, bdm[:, g0 - bcols : g1 - bcols],
                            op0=OP.add, op1=OP.add,
                        )
                        nc.sync.dma_start(
                            out_w.ap()[blk, :, g0:g1], ut[H : H + S, :]
                        )

        def emit_xchg(blk):
            # halo refill: seams first (dep only on seam group, drained
            # first), then E1/E3 split in halves so early copies only depend
            # on early drains. DMAs spread across SP and ACT queues so their
            # issue+DGE latencies overlap.
            s, _, _ = block_tiles[blk]
            lastw = nw - 1
            p0 = S * (nw - 1) - lat + P - H
            nc.sync.dma_start(
                s[0:H, 0:bb], s[seam : seam + H, lastw * bb : (lastw + 1) * bb]
            )
            nc.sync.dma_start(
                s[P - H : P, lastw * bb : (lastw + 1) * bb],
                s[p0 : p0 + H, 0:bb],
            )
            mid = (ng // 2) * (group // bb) * bb  # col split point
            if mid == 0 or mid >= (nw - 1) * bb:
                mid = None
            if mid:
                nc.sync.dma_start(
                    s[0:H, bb : mid + bb], s[P - 2 * H : P - H, 0:mid]
                )
                nc.sync.dma_start(
                    s[P - H : P, 0:mid], s[H : 2 * H, bb : mid + bb]
                )
                nc.sync.dma_start(
                    s[0:H, mid + bb : nw * bb],
                    s[P - 2 * H : P - H, mid : (nw - 1) * bb],
                )
                nc.sync.dma_start(
                    s[P - H : P, mid : (nw - 1) * bb],
                    s[H : 2 * H, mid + bb : nw * bb],
                )
            else:
                nc.sync.dma_start(
                    s[0:H, bb : nw * bb],
                    s[P - 2 * H : P - H, 0 : (nw - 1) * bb],
                )
                nc.sync.dma_start(
                    s[P - H : P, 0 : (nw - 1) * bb],
                    s[H : 2 * H, bb : nw * bb],
                )

        # Paired-block interleave: steps of blocks (2k, 2k+1) alternate so
        # one block's halo-exchange barrier (and per-group dependency cycles)
        # are hidden under the other block's compute.
        assert nblk % 2 == 0
        load_block(0)
        load_block(1)
        for pair in range(nblk // 2):
            a, b = 2 * pair, 2 * pair + 1
            if 2 * pair + 2 < nblk:
                load_block(2 * pair + 2)  # slot free: block 2*pair-2 is done
            for t in range(steps):
                emit_step(a, t)
                if (t + 1) % H == 0 and t != steps - 1:
                    emit_xchg(a)
                emit_step(b, t)
                if (t + 1) % H == 0 and t != steps - 1:
                    emit_xchg(b)
            if 2 * pair + 3 < nblk:
                load_block(2 * pair + 3)  # after pair's stores are queued
            del block_tiles[a], block_tiles[b]

    nc.compile()
    return nc


def _round_f32r_grid(x):
    """Round float32 array to the f32r grid (11 mantissa bits, round-half-up).
    Any value with the low 12 mantissa bits zero is exactly representable, so
    the device DMA's own f32r rounding is then the identity."""
    u = np.ascontiguousarray(x, dtype=np.float32).view(np.uint32)
    u2 = (u + np.uint32(1 << 11)) & np.uint32(0xFFFFF000)
    return u2.view(np.float32)


def _host_constants(K):
    K = np.asarray(K, dtype=np.float64)
    q0 = (1.0 - BETA) * EPS * K[0] * R
    q1 = (1.0 - BETA) * (1.0 - EPS + EPS * K[1]) * R
    q2 = (1.0 - BETA) * EPS * K[2] * R
    W = np.zeros((P, P), np.float32)
    for p in range(1, P - 1):
        W[p - 1, p] = -q0
        W[p, p] = -q1
        W[p + 1, p] = -q2
    D = np.zeros((P, P), np.float32)
    np.fill_diagonal(D, 1.0)
    csum = 0.25 * (q0 + q1 + q2) - 0.5
    consts = np.empty((P, 2), np.float32)
    consts[:, 0] = csum
    consts[:, 1] = csum + 0.5
    return W, D, consts


def _window(d, lat, bb, nblk):
    """[rows, lat] -> [nblk, P, nw*bb] with halo windows (stride S)."""
    nw = _nw(lat)
    c_idx = np.arange(nw) * S
    p_idx = np.arange(P)
    idx = (c_idx[:, None] + p_idx[None, :] - H) % lat  # [nw, P]
    win = d[:, idx]  # [rows, nw, P]
    win = win.reshape(nblk, bb, nw, P).transpose(0, 3, 2, 1)  # [nblk, P, nw, bb]
    return np.ascontiguousarray(win).reshape(nblk, P, nw * bb)


def _unwindow(o, lat, bb, nblk):
    """[nblk, S, nw*bb] -> [rows, lat]."""
    nw = _nw(lat)
    o = o.reshape(nblk, S, nw, bb).transpose(0, 3, 2, 1)  # [nblk, bb, nw, S]
    o = o.reshape(nblk * bb, nw * S)
    return o[:, :lat]


_NC_CACHE = {}
TRACE = False
LAST_RESULT = None


def _get_nc(lat, bb, nblk, steps):
    key = (lat, bb, nblk, steps)
    if key not in _NC_CACHE:
        _NC_CACHE[key] = build_nc(lat, bb, nblk, steps)
    return _NC_CACHE[key]


def kernel(drive, K):
    drive = np.asarray(drive, dtype=np.float32)
    K = np.asarray(K, dtype=np.float32)
    b, mid, lat = drive.shape
    d2 = drive.reshape(b, lat)
    W, D, consts = _host_constants(K)
    csum = np.float64(consts[0, 0])
    nc = _get_nc(LAT, BB, NBLK, STEPS)

    in_maps = []
    for c in range(N_CORES):
        dcore = d2[c * BPC : (c + 1) * BPC]
        dw = _window(dcore, LAT, BB, NBLK).astype(np.float64)
        s0 = ((dw - 0.5) ** 2).astype(np.float32)
        bdm = (BETA * dw[:, :, BCOLS:] + csum).astype(np.float32)
        bd = BETA * dw[:, :, :BCOLS]
        hi = _round_f32r_grid(bd.astype(np.float32))
        lo = (bd - hi.astype(np.float64)).astype(np.float32)
        drv_b = np.concatenate([hi, lo], axis=2)
        in_maps.append(
            {
                "s0_w": s0,
                "bdm_w": bdm,
                "drv_b": drv_b,
                "wmat": W,
                "dmat": D,
                "consts": consts,
            }
        )
    global LAST_RESULT
    res = None
    for attempt in range(3):
        try:
            res = run_bass_kernel_spmd(
                nc, in_maps, core_ids=list(range(N_CORES)), trace=TRACE
            )
            break
        except Exception:
            # transient NRT device errors have been observed; reset the jax
            # backend (forces a fresh PJRT client / device session) and retry
            if attempt == 2:
                raise
            import time

            try:
                import jax

                jax.clear_caches()
                from jax._src import xla_bridge

                xla_bridge._clear_backends()
            except Exception:
                pass
            time.sleep(5.0)
    LAST_RESULT = res
    outs = [_unwindow(res.results[c]["out_w"], LAT, BB, NBLK) for c in range(N_CORES)]
    out = np.concatenate(outs, axis=0).reshape(b, mid, lat)
    return np.clip(out, CLIP_LO, CLIP_HI).astype(np.float32)
